# revision 21
# baseline (speedup 1.0000x reference)
"""Trainium2 Bass kernel for a dense transformer attention layer.

Reference computation (per batch b of B=32, T=256 tokens, D=2048, 16 heads x 128):
    q = x @ wq.T ; k = x @ wk.T ; v = x @ wv.T        (torch Linear convention)
    q, k = rope(q), rope(k)
    attn = softmax(mask(q k^T / sqrt(128)))
    out  = (attn @ v) @ wo.T

Strategy: pure data parallelism over the batch dim - 4 batches per core on 8
NeuronCores, weights replicated, no collectives.  All tensors stay resident in
SBUF between phases (no DRAM round trips):
  phase 1: Q^T, K^T feature-major with RoPE fused into the PSUM drain
           (DVE muls + GpSimd add, signed sin table), 4+4 PSUM double buffer.
           V token-major via x-stationary matmuls.
  phase 2: per (batch, head): S^T = K^T.T Q^T with additive causal mask
           accumulated on the PE (identity x trineg matmul), one fused
           exp over [128,384] on ACT, denominators via ones-matmul,
           reciprocal_approx_fast + gpsimd broadcast, O^T = V^T P^T,
           normalize on the DVE drain.  Software-pipelined with lag 2.
  phase 3: y = O^T.T @ wo^T streamed from SBUF-resident O^T.
"""

import sys

if "/opt/trn_rl_repo" not in sys.path:
    sys.path.insert(0, "/opt/trn_rl_repo")

import numpy as np

B, T, D = 32, 256, 2048
H, HD = 16, 128
NCORES = 8
BLOC = B // NCORES          # batches per core = 4
TLOC = BLOC * T             # tokens per core = 1024
ROPE_BASE = 10000.0
SCALE = 1.0 / float(np.sqrt(HD))
NEG = -30000.0

_CACHE = {}


def _build():
    import concourse.tile as tile
    from concourse import bacc, mybir
    from contextlib import ExitStack

    F32 = mybir.dt.float32
    BF16 = mybir.dt.bfloat16
    Exp = mybir.ActivationFunctionType.Exp

    nc = bacc.Bacc("TRN2", target_bir_lowering=False)

    xT = nc.declare_dram_parameter("xT", [D, TLOC], BF16, isOutput=False)
    wqT = nc.declare_dram_parameter("wqT", [D, D], BF16, isOutput=False)
    wkT = nc.declare_dram_parameter("wkT", [D, D], BF16, isOutput=False)
    wvT = nc.declare_dram_parameter("wvT", [D, D], BF16, isOutput=False)
    woT = nc.declare_dram_parameter("woT", [D, D], BF16, isOutput=False)
    cosT = nc.declare_dram_parameter("cosT", [HD, 512], BF16, isOutput=False)
    sinTs = nc.declare_dram_parameter("sinTs", [HD, 512], BF16, isOutput=False)
    trineg = nc.declare_dram_parameter("trineg", [128, 128], BF16, isOutput=False)
    ident = nc.declare_dram_parameter("ident", [128, 128], BF16, isOutput=False)
    onesd = nc.declare_dram_parameter("onesd", [128, 1], BF16, isOutput=False)
    y = nc.declare_dram_parameter("y", [TLOC, D], F32, isOutput=True)

    with ExitStack() as ctx:
        tc = ctx.enter_context(tile.TileContext(nc))
        # resident pools
        xpool = ctx.enter_context(tc.tile_pool(name="xpool", bufs=1))
        qpool = ctx.enter_context(tc.tile_pool(name="qpool", bufs=1))
        kpool = ctx.enter_context(tc.tile_pool(name="kpool", bufs=1))
        vpool = ctx.enter_context(tc.tile_pool(name="vpool", bufs=1))
        opool = ctx.enter_context(tc.tile_pool(name="opool", bufs=1))
        consts = ctx.enter_context(tc.tile_pool(name="consts", bufs=1))
        # streaming pools
        wst = ctx.enter_context(tc.tile_pool(name="wst", bufs=8))
        rtmp = ctx.enter_context(tc.tile_pool(name="rtmp", bufs=2))
        att = ctx.enter_context(tc.tile_pool(name="att", bufs=2))
        drain = ctx.enter_context(tc.tile_pool(name="drain", bufs=3))

        # ---------- resident loads ----------
        # Interleave x-tile DMAs with the first projection group's weight
        # tiles so the PE can start at ~2us instead of waiting for the whole
        # serialized DMA-issue stream.
        xt_sb = []
        wt_pre = []
        for c in range(16):
            t_ = xpool.tile([128, TLOC], BF16, tag=f"xt{c}", name=f"xt{c}")
            nc.sync.dma_start(out=t_[:, 0:512],
                              in_=xT[c * 128:(c + 1) * 128, 0:512])
            nc.gpsimd.dma_start(out=t_[:, 512:1024],
                                in_=xT[c * 128:(c + 1) * 128, 512:1024])
            xt_sb.append(t_)
            w_ = wst.tile([128, 512], BF16, tag="wst", name="wt", bufs=16)
            nc.scalar.dma_start(out=w_, in_=wqT[c * 128:(c + 1) * 128, 0:512])
            wt_pre.append(w_)

        cos_sb = consts.tile([HD, 512], BF16, tag="cos", name="cos_sb")
        nc.sync.dma_start(out=cos_sb, in_=cosT[:, :])
        sin_sb = consts.tile([HD, 512], BF16, tag="sin", name="sin_sb")
        nc.sync.dma_start(out=sin_sb, in_=sinTs[:, :])
        tri_sb = consts.tile([128, 128], BF16, tag="tri", name="tri_sb")
        nc.sync.dma_start(out=tri_sb, in_=trineg[:, :])
        id_sb = consts.tile([128, 128], BF16, tag="id", name="id_sb")
        nc.sync.dma_start(out=id_sb, in_=ident[:, :])
        ones_col = consts.tile([128, 1], BF16, tag="ones", name="ones_col")
        nc.sync.dma_start(out=ones_col, in_=onesd[:, :])

        qt_sb = [qpool.tile([HD, TLOC], BF16, tag=f"qt{h}", name=f"qt{h}")
                 for h in range(H)]
        kt_sb = [kpool.tile([HD, TLOC], BF16, tag=f"kt{h}", name=f"kt{h}")
                 for h in range(H)]
        v_sb = [vpool.tile([128, D], BF16, tag=f"vt{t}", name=f"vt{t}")
                for t in range(8)]
        ot_sb = [opool.tile([HD, TLOC], BF16, tag=f"ot{h}", name=f"ot{h}")
                 for h in range(H)]

        # ---------- phase 1: Q^T / K^T projections with fused RoPE ----------
        def rope_drain(psum, dst, tb):
            # psum [128(head dims), 512(tokens)] -> dst[:, tb*512:...] bf16
            # with rotary applied.  sin table is sign-baked: rows 0:64 hold
            # -sin, rows 64:128 hold +sin, so the combine is a uniform add.
            tmp = rtmp.tile([128, 512], BF16, tag="rt_a", name="rt_a")
            tmp2 = rtmp.tile([128, 512], BF16, tag="rt_b", name="rt_b")
            nc.vector.tensor_mul(tmp[:, :], psum[:, :], cos_sb[:, :])
            nc.vector.tensor_mul(tmp2[0:64, :], psum[64:128, :], sin_sb[0:64, :])
            nc.vector.tensor_mul(tmp2[64:128, :], psum[0:64, :], sin_sb[64:128, :])
            nc.gpsimd.tensor_add(dst[:, tb * 512:(tb + 1) * 512],
                                 tmp[:, :], tmp2[:, :])

        ps1cm = tc.tile_pool(name="ps1", bufs=8, space="PSUM")
        ps = ps1cm.__enter__()
        for wparam, dst_sb in ((wqT, qt_sb), (wkT, kt_sb)):
            for ob in range(4):
                if wparam is wqT and ob == 0:
                    wts = wt_pre
                else:
                    wts = []
                    for c in range(16):
                        wt = wst.tile([128, 512], BF16, tag="wst", name="wt",
                                      bufs=16)
                        dq = nc.sync if wparam is wqT else nc.scalar
                        dq.dma_start(
                            out=wt,
                            in_=wparam[c * 128:(c + 1) * 128,
                                       ob * 512:(ob + 1) * 512])
                        wts.append(wt)
                for half in range(2):
                    psums = [[ps.tile([128, 512], F32, tag="ps", name="pqk")
                              for _ in range(2)] for _ in range(2)]
                    for c in range(16):
                        for oi in range(2):
                            c0 = half * 256 + oi * 128
                            for tb in range(2):
                                nc.tensor.matmul(
                                    psums[oi][tb][:, :],
                                    wts[c][:, c0:c0 + 128],
                                    xt_sb[c][:, tb * 512:(tb + 1) * 512],
                                    start=(c == 0), stop=(c == 15))
                    for oi in range(2):
                        h_idx = ob * 4 + half * 2 + oi
                        for tb in range(2):
                            rope_drain(psums[oi][tb], dst_sb[h_idx], tb)

        # ---------- phase 1b: V token-major ----------
        for ob in range(4):
            psums = [ps.tile([128, 512], F32, tag="ps", name="pv")
                     for _ in range(8)]
            for c in range(16):
                wt = wst.tile([128, 512], BF16, tag="wstv", name="wtv", bufs=6)
                nc.sync.dma_start(
                    out=wt,
                    in_=wvT[c * 128:(c + 1) * 128, ob * 512:(ob + 1) * 512])
                for t in range(8):
                    nc.tensor.matmul(
                        psums[t][:, :],
                        xt_sb[c][:, t * 128:(t + 1) * 128],
                        wt[:, :],
                        start=(c == 0), stop=(c == 15))
            for t in range(8):
                if ob == 3 and t % 2 == 1:
                    nc.scalar.copy(
                        v_sb[t][:, ob * 512:(ob + 1) * 512], psums[t][:, :])
                else:
                    nc.vector.tensor_copy(
                        v_sb[t][:, ob * 512:(ob + 1) * 512], psums[t][:, :])
        ps1cm.__exit__(None, None, None)

        # ---------- phase 2: attention, software pipelined ----------
        ps2cm = tc.tile_pool(name="ps2", bufs=2, space="PSUM")
        ps = ps2cm.__enter__()
        pairs = [(b, h) for b in range(BLOC) for h in range(H)]
        st_of = {}
        pt_of = {}
        bc_of = {}
        otp_of = {}

        def stage1(i):
            b, h = pairs[i]
            t0 = b * T
            st = ps.tile([128, 384], F32, tag="st", name="st", bufs=2)
            nc.tensor.matmul(st[:, 0:256],
                             kt_sb[h][:, t0:t0 + 128],
                             qt_sb[h][:, t0:t0 + 256],
                             start=True, stop=False)
            nc.tensor.matmul(st[:, 256:384],
                             kt_sb[h][:, t0 + 128:t0 + 256],
                             qt_sb[h][:, t0 + 128:t0 + 256],
                             start=False, stop=False, skip_group_check=True)
            nc.tensor.matmul(st[:, 0:128], id_sb[:, :], tri_sb[:, :],
                             start=False, stop=False, skip_group_check=True)
            nc.tensor.matmul(st[:, 256:384], id_sb[:, :], tri_sb[:, :],
                             start=False, stop=True, skip_group_check=True)
            pt = att.tile([128, 384], BF16, tag="pt", name="pt", bufs=3)
            nc.scalar.activation(pt[:, :], st[:, :], Exp, scale=SCALE)
            pt_of[i] = pt

        def stage2a(i):
            b, h = pairs[i]
            pt = pt_of[i]
            dps = ps.tile([1, 256], F32, tag="dps", name="dps", bufs=2)
            nc.tensor.matmul(dps[:, 0:256], ones_col[:, :], pt[:, 0:256],
                             start=True, stop=False)
            nc.tensor.matmul(dps[:, 128:256], ones_col[:, :], pt[:, 256:384],
                             start=False, stop=True, skip_group_check=True)
            rd = att.tile([1, 256], F32, tag="rd", name="rd", bufs=2)
            nc.vector.reciprocal_approx_fast(out=rd[:, :], in_=dps[:, :])
            bc = att.tile([128, 256], F32, tag="bc", name="bc", bufs=2)
            nc.gpsimd.partition_broadcast(bc[:, :], rd[:, :])
            bc_of[i] = bc
            otp = ps.tile([128, 256], F32, tag="otp", name="otp", bufs=2)
            nc.tensor.matmul(otp[:, 0:256],
                             v_sb[2 * b][:, h * 128:(h + 1) * 128],
                             pt[:, 0:256], start=True, stop=False)
            nc.tensor.matmul(otp[:, 128:256],
                             v_sb[2 * b + 1][:, h * 128:(h + 1) * 128],
                             pt[:, 256:384],
                             start=False, stop=True, skip_group_check=True)
            otp_of[i] = otp

        def stage2b(i):
            b, h = pairs[i]
            nc.vector.tensor_mul(ot_sb[h][:, b * T:(b + 1) * T],
                                 otp_of[i][:, :], bc_of[i][:, :])

        n = len(pairs)
        for i in range(n + 2):
            if i < n:
                stage1(i)
            if 1 <= i <= n:
                stage2a(i - 1)
            if i >= 2:
                stage2b(i - 2)

        ps2cm.__exit__(None, None, None)

        # ---------- phase 3: output projection ----------
        ps3cm = tc.tile_pool(name="ps3", bufs=8, space="PSUM")
        ps = ps3cm.__enter__()
        for mb in range(4):
            psums = [ps.tile([128, 512], F32, tag="ps", name="py")
                     for _ in range(8)]
            for e in range(16):
                wt = wst.tile([128, 512], BF16, tag="wsto", name="wo_t",
                              bufs=7)
                nc.sync.dma_start(
                    out=wt,
                    in_=woT[e * 128:(e + 1) * 128, mb * 512:(mb + 1) * 512])
                for t in range(8):
                    nc.tensor.matmul(
                        psums[t][:, :],
                        ot_sb[e][:, t * 128:(t + 1) * 128],
                        wt[:, :],
                        start=(e == 0), stop=(e == 15))
            for t in range(8):
                ysb = drain.tile([128, 512], F32, tag="drain_y", name="ysb")
                nc.vector.tensor_copy(ysb[:, :], psums[t][:, :])
                nc.gpsimd.dma_start(
                    out=y[t * 128:(t + 1) * 128, mb * 512:(mb + 1) * 512],
                    in_=ysb[:, :])
        ps3cm.__exit__(None, None, None)

    nc.compile()
    return nc


def _host_prep(x, mask, wq, wk, wv, wo):
    import ml_dtypes
    f32 = np.float32
    bf16 = ml_dtypes.bfloat16
    wqT = np.ascontiguousarray(np.asarray(wq, f32).T.astype(bf16))
    wkT = np.ascontiguousarray(np.asarray(wk, f32).T.astype(bf16))
    wvT = np.ascontiguousarray(np.asarray(wv, f32).T.astype(bf16))
    woT = np.ascontiguousarray(np.asarray(wo, f32).T.astype(bf16))

    inv_freq = (1.0 / (ROPE_BASE ** (np.arange(0, HD, 2, dtype=f32) / HD))).astype(f32)
    t_ = np.arange(T, dtype=f32)
    freqs = np.outer(t_, inv_freq)                    # [T, 64]
    emb = np.concatenate([freqs, freqs], axis=-1)     # [T, 128]
    cosT = np.ascontiguousarray(np.cos(emb).astype(f32).T)   # [128, T]
    sinT = np.ascontiguousarray(np.sin(emb).astype(f32).T)
    sinT[0:64, :] = -sinT[0:64, :]                    # sign-baked
    cosT = np.tile(cosT, (1, 512 // T)).astype(bf16)  # [128, 512]
    sinTs = np.tile(sinT, (1, 512 // T)).astype(bf16)

    # trineg[k, q] = NEG where key k (partition) > query q (column), else 0
    kk = np.arange(128)[:, None]
    qq = np.arange(128)[None, :]
    trineg = np.where(kk > qq, np.float32(NEG), np.float32(0.0)).astype(bf16)
    ident = np.eye(128, dtype=f32).astype(bf16)
    onesd = np.ones((128, 1), bf16)

    shared = dict(wqT=wqT, wkT=wkT, wvT=wvT, woT=woT, cosT=cosT, sinTs=sinTs,
                  trineg=np.ascontiguousarray(trineg),
                  ident=np.ascontiguousarray(ident), onesd=onesd)
    xf = np.asarray(x, f32)
    in_maps = []
    for i in range(NCORES):
        xs = xf[i * BLOC:(i + 1) * BLOC].reshape(TLOC, D)
        m = dict(shared)
        m["xT"] = np.ascontiguousarray(xs.T.astype(bf16))
        in_maps.append(m)
    return in_maps


def _run(x, mask, wq, wk, wv, wo, trace=False):
    from concourse.bass_utils import run_bass_kernel_spmd

    if "nc" not in _CACHE:
        _CACHE["nc"] = _build()
    nc = _CACHE["nc"]
    in_maps = _host_prep(x, mask, wq, wk, wv, wo)
    res = run_bass_kernel_spmd(nc, in_maps, core_ids=list(range(NCORES)),
                               trace=trace)
    out = np.empty((B, T, D), np.float32)
    for i in range(NCORES):
        out[i * BLOC:(i + 1) * BLOC] = res.results[i]["y"].reshape(BLOC, T, D)
    return out, res


def kernel(x, mask, wq, wk, wv, wo):
    out, _ = _run(x, mask, wq, wk, wv, wo, trace=False)
    return out


# revision 22
# speedup vs baseline: 1.0167x; 1.0167x over previous
"""Trainium2 Bass kernel for a dense transformer attention layer.

Reference computation (per batch b of B=32, T=256 tokens, D=2048, 16 heads x 128):
    q = x @ wq.T ; k = x @ wk.T ; v = x @ wv.T        (torch Linear convention)
    q, k = rope(q), rope(k)
    attn = softmax(mask(q k^T / sqrt(128)))
    out  = (attn @ v) @ wo.T

Strategy: pure data parallelism over the batch dim - 4 batches per core on 8
NeuronCores, weights replicated, no collectives.  All tensors stay resident in
SBUF between phases (no DRAM round trips):
  phase 1: Q^T, K^T feature-major with RoPE fused into the PSUM drain
           (DVE muls + GpSimd add, signed sin table), 4+4 PSUM double buffer.
           V token-major via x-stationary matmuls.
  phase 2: per (batch, head): S^T = K^T.T Q^T with additive causal mask
           accumulated on the PE (identity x trineg matmul), one fused
           exp over [128,384] on ACT, denominators via ones-matmul,
           reciprocal_approx_fast + gpsimd broadcast, O^T = V^T P^T,
           normalize on the DVE drain.  Software-pipelined with lag 2.
  phase 3: y = O^T.T @ wo^T streamed from SBUF-resident O^T.
"""

import sys

if "/opt/trn_rl_repo" not in sys.path:
    sys.path.insert(0, "/opt/trn_rl_repo")

import numpy as np

B, T, D = 32, 256, 2048
H, HD = 16, 128
NCORES = 8
BLOC = B // NCORES          # batches per core = 4
TLOC = BLOC * T             # tokens per core = 1024
ROPE_BASE = 10000.0
SCALE = 1.0 / float(np.sqrt(HD))
NEG = -30000.0

_CACHE = {}


def _build():
    import concourse.tile as tile
    from concourse import bacc, mybir
    from contextlib import ExitStack

    F32 = mybir.dt.float32
    BF16 = mybir.dt.bfloat16
    Exp = mybir.ActivationFunctionType.Exp

    nc = bacc.Bacc("TRN2", target_bir_lowering=False)

    xT = nc.declare_dram_parameter("xT", [D, TLOC], BF16, isOutput=False)
    wqT = nc.declare_dram_parameter("wqT", [D, D], BF16, isOutput=False)
    wkT = nc.declare_dram_parameter("wkT", [D, D], BF16, isOutput=False)
    wvT = nc.declare_dram_parameter("wvT", [D, D], BF16, isOutput=False)
    woT = nc.declare_dram_parameter("woT", [D, D], BF16, isOutput=False)
    cosT = nc.declare_dram_parameter("cosT", [HD, 512], BF16, isOutput=False)
    sinTs = nc.declare_dram_parameter("sinTs", [HD, 512], BF16, isOutput=False)
    trineg = nc.declare_dram_parameter("trineg", [128, 128], BF16, isOutput=False)
    ident = nc.declare_dram_parameter("ident", [128, 128], BF16, isOutput=False)
    onesd = nc.declare_dram_parameter("onesd", [128, 1], BF16, isOutput=False)
    y = nc.declare_dram_parameter("y", [TLOC, D], F32, isOutput=True)

    with ExitStack() as ctx:
        tc = ctx.enter_context(tile.TileContext(nc))
        # resident pools
        xpool = ctx.enter_context(tc.tile_pool(name="xpool", bufs=1))
        qpool = ctx.enter_context(tc.tile_pool(name="qpool", bufs=1))
        kpool = ctx.enter_context(tc.tile_pool(name="kpool", bufs=1))
        vpool = ctx.enter_context(tc.tile_pool(name="vpool", bufs=1))
        opool = ctx.enter_context(tc.tile_pool(name="opool", bufs=1))
        consts = ctx.enter_context(tc.tile_pool(name="consts", bufs=1))
        # streaming pools
        wst = ctx.enter_context(tc.tile_pool(name="wst", bufs=8))
        rtmp = ctx.enter_context(tc.tile_pool(name="rtmp", bufs=2))
        att = ctx.enter_context(tc.tile_pool(name="att", bufs=2))
        drain = ctx.enter_context(tc.tile_pool(name="drain", bufs=3))

        # ---------- resident loads ----------
        # Interleave x-tile DMAs with the first projection group's weight
        # tiles so the PE can start at ~2us instead of waiting for the whole
        # serialized DMA-issue stream.
        xt_sb = []
        wt_pre = []
        for c in range(16):
            t_ = xpool.tile([128, TLOC], BF16, tag=f"xt{c}", name=f"xt{c}")
            nc.sync.dma_start(out=t_, in_=xT[c * 128:(c + 1) * 128, :])
            xt_sb.append(t_)
            w_ = wst.tile([128, 512], BF16, tag="wst", name="wt", bufs=16)
            nc.scalar.dma_start(out=w_, in_=wqT[c * 128:(c + 1) * 128, 0:512])
            wt_pre.append(w_)

        cos_sb = consts.tile([HD, 512], BF16, tag="cos", name="cos_sb")
        nc.sync.dma_start(out=cos_sb, in_=cosT[:, :])
        sin_sb = consts.tile([HD, 512], BF16, tag="sin", name="sin_sb")
        nc.sync.dma_start(out=sin_sb, in_=sinTs[:, :])
        tri_sb = consts.tile([128, 128], BF16, tag="tri", name="tri_sb")
        nc.sync.dma_start(out=tri_sb, in_=trineg[:, :])
        id_sb = consts.tile([128, 128], BF16, tag="id", name="id_sb")
        nc.sync.dma_start(out=id_sb, in_=ident[:, :])
        ones_col = consts.tile([128, 1], BF16, tag="ones", name="ones_col")
        nc.sync.dma_start(out=ones_col, in_=onesd[:, :])

        qt_sb = [qpool.tile([HD, TLOC], BF16, tag=f"qt{h}", name=f"qt{h}")
                 for h in range(H)]
        kt_sb = [kpool.tile([HD, TLOC], BF16, tag=f"kt{h}", name=f"kt{h}")
                 for h in range(H)]
        v_sb = [vpool.tile([128, D], BF16, tag=f"vt{t}", name=f"vt{t}")
                for t in range(8)]
        ot_sb = [opool.tile([HD, TLOC], BF16, tag=f"ot{h}", name=f"ot{h}")
                 for h in range(H)]

        # ---------- phase 1: Q^T / K^T projections with fused RoPE ----------
        def rope_drain(psum, dst, tb):
            # psum [128(head dims), 512(tokens)] -> dst[:, tb*512:...] bf16
            # with rotary applied.  sin table is sign-baked: rows 0:64 hold
            # -sin, rows 64:128 hold +sin, so the combine is a uniform add.
            tmp = rtmp.tile([128, 512], BF16, tag="rt_a", name="rt_a")
            tmp2 = rtmp.tile([128, 512], BF16, tag="rt_b", name="rt_b")
            nc.vector.tensor_mul(tmp[:, :], psum[:, :], cos_sb[:, :])
            nc.vector.tensor_mul(tmp2[0:64, :], psum[64:128, :], sin_sb[0:64, :])
            nc.vector.tensor_mul(tmp2[64:128, :], psum[0:64, :], sin_sb[64:128, :])
            nc.gpsimd.tensor_add(dst[:, tb * 512:(tb + 1) * 512],
                                 tmp[:, :], tmp2[:, :])

        ps1cm = tc.tile_pool(name="ps1", bufs=8, space="PSUM")
        ps = ps1cm.__enter__()
        for wparam, dst_sb in ((wqT, qt_sb), (wkT, kt_sb)):
            for ob in range(4):
                if wparam is wqT and ob == 0:
                    wts = wt_pre
                else:
                    wts = []
                    for c in range(16):
                        wt = wst.tile([128, 512], BF16, tag="wst", name="wt",
                                      bufs=16)
                        dq = nc.sync if wparam is wqT else nc.scalar
                        dq.dma_start(
                            out=wt,
                            in_=wparam[c * 128:(c + 1) * 128,
                                       ob * 512:(ob + 1) * 512])
                        wts.append(wt)
                for half in range(2):
                    psums = [[ps.tile([128, 512], F32, tag="ps", name="pqk")
                              for _ in range(2)] for _ in range(2)]
                    for c in range(16):
                        for oi in range(2):
                            c0 = half * 256 + oi * 128
                            for tb in range(2):
                                nc.tensor.matmul(
                                    psums[oi][tb][:, :],
                                    wts[c][:, c0:c0 + 128],
                                    xt_sb[c][:, tb * 512:(tb + 1) * 512],
                                    start=(c == 0), stop=(c == 15))
                    for oi in range(2):
                        h_idx = ob * 4 + half * 2 + oi
                        for tb in range(2):
                            rope_drain(psums[oi][tb], dst_sb[h_idx], tb)

        # ---------- phase 1b: V token-major ----------
        for ob in range(4):
            psums = [ps.tile([128, 512], F32, tag="ps", name="pv")
                     for _ in range(8)]
            for c in range(16):
                wt = wst.tile([128, 512], BF16, tag="wstv", name="wtv", bufs=6)
                nc.sync.dma_start(
                    out=wt,
                    in_=wvT[c * 128:(c + 1) * 128, ob * 512:(ob + 1) * 512])
                for t in range(8):
                    nc.tensor.matmul(
                        psums[t][:, :],
                        xt_sb[c][:, t * 128:(t + 1) * 128],
                        wt[:, :],
                        start=(c == 0), stop=(c == 15))
            for t in range(8):
                if ob == 3 and t % 2 == 1:
                    nc.scalar.copy(
                        v_sb[t][:, ob * 512:(ob + 1) * 512], psums[t][:, :])
                else:
                    nc.vector.tensor_copy(
                        v_sb[t][:, ob * 512:(ob + 1) * 512], psums[t][:, :])
        ps1cm.__exit__(None, None, None)

        # ---------- phase 2: attention, software pipelined ----------
        ps2cm = tc.tile_pool(name="ps2", bufs=2, space="PSUM")
        ps = ps2cm.__enter__()
        pairs = [(b, h) for b in range(BLOC) for h in range(H)]
        st_of = {}
        pt_of = {}
        bc_of = {}
        otp_of = {}

        def stage1(i):
            b, h = pairs[i]
            t0 = b * T
            st = ps.tile([128, 384], F32, tag="st", name="st", bufs=2)
            nc.tensor.matmul(st[:, 0:256],
                             kt_sb[h][:, t0:t0 + 128],
                             qt_sb[h][:, t0:t0 + 256],
                             start=True, stop=False)
            nc.tensor.matmul(st[:, 256:384],
                             kt_sb[h][:, t0 + 128:t0 + 256],
                             qt_sb[h][:, t0 + 128:t0 + 256],
                             start=False, stop=False, skip_group_check=True)
            nc.tensor.matmul(st[:, 0:128], id_sb[:, :], tri_sb[:, :],
                             start=False, stop=False, skip_group_check=True)
            nc.tensor.matmul(st[:, 256:384], id_sb[:, :], tri_sb[:, :],
                             start=False, stop=True, skip_group_check=True)
            pt = att.tile([128, 384], BF16, tag="pt", name="pt", bufs=3)
            nc.scalar.activation(pt[:, :], st[:, :], Exp, scale=SCALE)
            pt_of[i] = pt

        def stage2a(i):
            b, h = pairs[i]
            pt = pt_of[i]
            dps = ps.tile([1, 256], F32, tag="dps", name="dps", bufs=2)
            nc.tensor.matmul(dps[:, 0:256], ones_col[:, :], pt[:, 0:256],
                             start=True, stop=False)
            nc.tensor.matmul(dps[:, 128:256], ones_col[:, :], pt[:, 256:384],
                             start=False, stop=True, skip_group_check=True)
            rd = att.tile([1, 256], F32, tag="rd", name="rd", bufs=2)
            nc.vector.reciprocal_approx_fast(out=rd[:, :], in_=dps[:, :])
            bc = att.tile([128, 256], F32, tag="bc", name="bc", bufs=2)
            nc.gpsimd.partition_broadcast(bc[:, :], rd[:, :])
            bc_of[i] = bc
            otp = ps.tile([128, 256], F32, tag="otp", name="otp", bufs=2)
            nc.tensor.matmul(otp[:, 0:256],
                             v_sb[2 * b][:, h * 128:(h + 1) * 128],
                             pt[:, 0:256], start=True, stop=False)
            nc.tensor.matmul(otp[:, 128:256],
                             v_sb[2 * b + 1][:, h * 128:(h + 1) * 128],
                             pt[:, 256:384],
                             start=False, stop=True, skip_group_check=True)
            otp_of[i] = otp

        def stage2b(i):
            b, h = pairs[i]
            nc.vector.tensor_mul(ot_sb[h][:, b * T:(b + 1) * T],
                                 otp_of[i][:, :], bc_of[i][:, :])

        n = len(pairs)
        for i in range(n + 2):
            if i < n:
                stage1(i)
            if 1 <= i <= n:
                stage2a(i - 1)
            if i >= 2:
                stage2b(i - 2)

        ps2cm.__exit__(None, None, None)

        # ---------- phase 3: output projection ----------
        ps3cm = tc.tile_pool(name="ps3", bufs=8, space="PSUM")
        ps = ps3cm.__enter__()
        for mb in range(4):
            psums = [ps.tile([128, 512], F32, tag="ps", name="py")
                     for _ in range(8)]
            for e in range(16):
                wt = wst.tile([128, 512], BF16, tag="wsto", name="wo_t",
                              bufs=7)
                nc.sync.dma_start(
                    out=wt,
                    in_=woT[e * 128:(e + 1) * 128, mb * 512:(mb + 1) * 512])
                for t in range(8):
                    nc.tensor.matmul(
                        psums[t][:, :],
                        ot_sb[e][:, t * 128:(t + 1) * 128],
                        wt[:, :],
                        start=(e == 0), stop=(e == 15))
            for t in range(8):
                ysb = drain.tile([128, 512], F32, tag="drain_y", name="ysb")
                if mb == 3 and t % 2 == 1:
                    nc.scalar.copy(ysb[:, :], psums[t][:, :])
                else:
                    nc.vector.tensor_copy(ysb[:, :], psums[t][:, :])
                nc.gpsimd.dma_start(
                    out=y[t * 128:(t + 1) * 128, mb * 512:(mb + 1) * 512],
                    in_=ysb[:, :])
        ps3cm.__exit__(None, None, None)

    nc.compile()
    return nc


def _host_prep(x, mask, wq, wk, wv, wo):
    import ml_dtypes
    f32 = np.float32
    bf16 = ml_dtypes.bfloat16
    wqT = np.ascontiguousarray(np.asarray(wq, f32).T.astype(bf16))
    wkT = np.ascontiguousarray(np.asarray(wk, f32).T.astype(bf16))
    wvT = np.ascontiguousarray(np.asarray(wv, f32).T.astype(bf16))
    woT = np.ascontiguousarray(np.asarray(wo, f32).T.astype(bf16))

    inv_freq = (1.0 / (ROPE_BASE ** (np.arange(0, HD, 2, dtype=f32) / HD))).astype(f32)
    t_ = np.arange(T, dtype=f32)
    freqs = np.outer(t_, inv_freq)                    # [T, 64]
    emb = np.concatenate([freqs, freqs], axis=-1)     # [T, 128]
    cosT = np.ascontiguousarray(np.cos(emb).astype(f32).T)   # [128, T]
    sinT = np.ascontiguousarray(np.sin(emb).astype(f32).T)
    sinT[0:64, :] = -sinT[0:64, :]                    # sign-baked
    cosT = np.tile(cosT, (1, 512 // T)).astype(bf16)  # [128, 512]
    sinTs = np.tile(sinT, (1, 512 // T)).astype(bf16)

    # trineg[k, q] = NEG where key k (partition) > query q (column), else 0
    kk = np.arange(128)[:, None]
    qq = np.arange(128)[None, :]
    trineg = np.where(kk > qq, np.float32(NEG), np.float32(0.0)).astype(bf16)
    ident = np.eye(128, dtype=f32).astype(bf16)
    onesd = np.ones((128, 1), bf16)

    shared = dict(wqT=wqT, wkT=wkT, wvT=wvT, woT=woT, cosT=cosT, sinTs=sinTs,
                  trineg=np.ascontiguousarray(trineg),
                  ident=np.ascontiguousarray(ident), onesd=onesd)
    xf = np.asarray(x, f32)
    in_maps = []
    for i in range(NCORES):
        xs = xf[i * BLOC:(i + 1) * BLOC].reshape(TLOC, D)
        m = dict(shared)
        m["xT"] = np.ascontiguousarray(xs.T.astype(bf16))
        in_maps.append(m)
    return in_maps


def _run(x, mask, wq, wk, wv, wo, trace=False):
    from concourse.bass_utils import run_bass_kernel_spmd

    if "nc" not in _CACHE:
        _CACHE["nc"] = _build()
    nc = _CACHE["nc"]
    in_maps = _host_prep(x, mask, wq, wk, wv, wo)
    res = run_bass_kernel_spmd(nc, in_maps, core_ids=list(range(NCORES)),
                               trace=trace)
    out = np.empty((B, T, D), np.float32)
    for i in range(NCORES):
        out[i * BLOC:(i + 1) * BLOC] = res.results[i]["y"].reshape(BLOC, T, D)
    return out, res


def kernel(x, mask, wq, wk, wv, wo):
    out, _ = _run(x, mask, wq, wk, wv, wo, trace=False)
    return out
